# revision 17
# baseline (speedup 1.0000x reference)
"""Masked dot-product attention on 8 TRN2 NeuronCores (Bass/Tile).

Strategy (fixed problem shape B=16, NQ=NK=2048, D=DV=128):

* Work unit = one 128-key k-tile of one batch.  ceil(L_b/128) tiles per
  batch; tiles are distributed over 8 cores as SPMD "slots": every core
  runs S slots with fixed extents [e_0..e_{S-1}]; the host assigns each
  (core, slot) one contiguous (batch, k-range) segment at call time (an
  exact DFS packer minimizes executed tiles).  Segments of one batch may
  land on different cores/slots; the host sums the partial results.

* Device math per k-tile t, per q-half (1024 queries):
    S^T[k,q]  = K_tile @ Q^T            (bf16 in, fp32 PSUM)
    P^T[k,q]  = exp(S^T/sqrt(D)+bias_k) (one ScalarE op, bf16 out; bias
                is -1e6 for masked keys so P underflows to exactly 0)
    O^T[v,q] += V_tile^T-contraction    (bf16 matmul, fp32 PSUM accum)
    d[1,q]   += ones^T @ (P-quad-sums)  (P tiles pair+quad-summed on DVE;
                one ones-matmul per 4 tiles keeps the PE under the
                ScalarE exp roofline)
  bf16 everywhere on-device: fp16 hits slow paths on every engine
  (no FWL for LDWEIGHTS, ~20% slower matmul stream, slower ACT + DVE).

* All slot inputs ride ONE packed bf16 DRAM tensor [128, Q|K|V|bias]
  (single ~700ns DMA-issue instead of five); outputs o (fp16 numerator)
  and d (fp16 denominator) are per-slot; host accumulates fp32, divides.
"""

import math

import ml_dtypes
import numpy as np

import concourse.bass as bass  # noqa: F401
import concourse.mybir as mybir
import concourse.tile as tile
from concourse import bacc
from concourse.bass_utils import run_bass_kernel_spmd
from concourse.vector_clock import ScopedClock


class _FastExitTileContext(tile.TileContext):
    """TileContext whose exit skips the end-of-program semaphore recycling.

    The stock ``_drain_and_barrier`` emits a serialized clear of every
    allocated semaphore (~250 x ~35ns) plus a DMA-queue reset and a second
    all-engine barrier -- ~9us of pure epilogue on the measured critical
    path.  Those exist so the semaphores are reusable by a LATER kernel in
    the same NEFF execution or a re-execution of the same loaded NEFF;
    each kernel() call here loads a fresh NEFF (sems start zeroed), so the
    final drain + one barrier is all that is needed for correctness of
    this single-kernel program.
    """

    def _drain_and_barrier(self, tick_clock, wait_clock):
        drain_inst = self.nc.sync.drain()
        wait_clock.add_sem_waits(
            drain_inst.ins, ScopedClock({None: tick_clock.global_clock})
        )
        popped = self.nc._tile_sem_poison_stack.pop()
        assert popped is self._sem_poison

B, NQ, NK, D, DV = 16, 2048, 2048, 128, 128
NCORES = 8
KT = 128  # keys per k-tile (partition dim)
QH = 1024  # queries per q-half (PSUM sizing)
NEG = np.float32(-1.0e6)
SCALE = 1.0 / math.sqrt(D)

F32 = mybir.dt.float32
F16 = mybir.dt.float16
BF16 = mybir.dt.bfloat16

_PROGRAM_CACHE: dict[tuple, object] = {}
LAST_RESULT = None  # BassKernelResults of the most recent run (for test.py)


# ---------------------------------------------------------------- scheduling
def _gen_extents(total, maxpart, nleft, prefix):
    """Descending tuples of nleft positive ints summing to total."""
    if nleft == 1:
        if 1 <= total <= maxpart:
            yield prefix + [total]
        return
    lo = (total + nleft - 1) // nleft
    for e in range(min(maxpart, total - nleft + 1), lo - 1, -1):
        yield from _gen_extents(total - e, e, nleft - 1, prefix + [e])


def _pack_exact(sizes, bins, node_cap=200000):
    """Cut batches into contiguous parts placed one-per-bin (part<=bin).
    bins: descending extent list (8 copies per slot).  Returns list of
    (bin_idx, batch, n) or None.  DFS w/ memo on (bin_idx, rem-state)."""
    nb = len(bins)
    suffix = [0] * (nb + 1)
    for i in range(nb - 1, -1, -1):
        suffix[i] = suffix[i + 1] + bins[i]
    seen = set()
    out = []
    nodes = [0]

    def dfs(bi, rem):
        nodes[0] += 1
        if nodes[0] > node_cap:
            return False
        tot = sum(rem)
        if tot == 0:
            return True
        if bi == nb or tot > suffix[bi]:
            return False
        key = (bi, tuple(sorted(rem)))
        if key in seen:
            return False
        order = sorted(range(len(rem)), key=lambda b: -rem[b])
        tried = set()
        for b in order:
            r = rem[b]
            if r <= 0 or r in tried:
                continue
            tried.add(r)
            c = min(r, bins[bi])
            rem[b] -= c
            out.append((bi, b, c))
            if dfs(bi + 1, rem):
                return True
            out.pop()
            rem[b] += c
        if dfs(bi + 1, rem):  # leave bin empty
            return True
        seen.add(key)
        return False

    return list(out) if dfs(0, list(sizes)) else None


def _schedule(sizes, n_cores=NCORES):
    """Pick slot extents minimizing (executed tiles, slots); assign parts.
    Returns (extents descending, assign[core][slot] = (batch, t0, n)|None)."""
    total = sum(sizes)
    cap = (total + n_cores - 1) // n_cores
    best = None
    for tot in range(cap, cap + 4):
        for nslots in range(1, 6):
            if best is not None:
                break
            for ext in _gen_extents(tot, min(16, max(sizes)), nslots, []):
                bins = [e for e in ext for _ in range(n_cores)]
                parts = _pack_exact(sizes, bins)
                if parts is not None:
                    best = (ext, parts)
                    break
        if best is not None:
            break
    assert best is not None, "exact packer failed"
    ext, parts = best
    nslots = len(ext)
    assign = [[None] * nslots for _ in range(n_cores)]
    nxt = [0] * len(sizes)
    # DFS emits parts in bin order, so same-batch parts get increasing t0.
    for bi, b, n in sorted(parts, key=lambda p: p[0]):
        s, c = bi // n_cores, bi % n_cores
        assign[c][s] = (b, nxt[b], n)
        nxt[b] += n
    # ascending (big slot last -> short tail drain), but lead with the
    # second-smallest: slot0's compute must cover slot1+2's input DMAs
    order = sorted(range(nslots), key=lambda s: ext[s])
    if len(order) > 1:
        order[0], order[1] = order[1], order[0]
    ext_o = [ext[s] for s in order]
    assign_o = [[row[s] for s in order] for row in assign]
    return ext_o, assign_o


def _widths(e):
    """Packed input column layout: Q | K | V | bias (all bf16)."""
    return NQ, e * KT, e * DV, e


# ------------------------------------------------------------ device program
def _build(extents):
    # The Bass preamble clears the full 254-sem kernel range (serialized
    # sem-file sweep + NRT pseudo-barrier fence, ~3us).  Semaphores are
    # already zero at NEFF load, so skip it -- same argument as the
    # _FastExitTileContext epilogue skip.
    _g = bass.BassGpSimd
    _orig = (_g.sem_clear, _g.dma_reset, bass.Bass._nrt_pseudo_barrier)
    _g.sem_clear = lambda self, sem: None
    _g.dma_reset = lambda self, semaphore_range=None: None
    bass.Bass._nrt_pseudo_barrier = lambda self: None
    try:
        nc = bacc.Bacc()
    finally:
        _g.sem_clear, _g.dma_reset, bass.Bass._nrt_pseudo_barrier = _orig
    emax = max(extents)
    in_d, o_d, d_d = [], [], []
    for s, e in enumerate(extents):
        w = sum(_widths(e))
        in_d.append(nc.dram_tensor(f"in{s}", [D, w], BF16, kind="ExternalInput"))
        o_d.append(nc.dram_tensor(f"o{s}", [DV, NQ], F16, kind="ExternalOutput"))
        d_d.append(nc.dram_tensor(f"d{s}", [1, NQ], F16, kind="ExternalOutput"))

    wmax = sum(_widths(emax))
    with _FastExitTileContext(nc) as tc:
        with (
            tc.tile_pool(name="sb", bufs=2) as sb,
            tc.tile_pool(name="ps", bufs=1, space="PSUM") as ps,
        ):
            ones = sb.tile([KT, 1], BF16, tag="ones", bufs=1)
            nc.vector.memset(ones[:], 1.0)
            # warmup: exp ACT-table load + PE HAM ramp during input DMA
            wsrc = sb.tile([KT, 512], BF16, tag="warm", bufs=1)
            nc.vector.memset(wsrc[:], 0.0)
            wpt = sb.tile([KT, QH], BF16, tag="pt", bufs=8)
            nc.scalar.activation(
                wpt[:, :512], wsrc[:], mybir.ActivationFunctionType.Exp
            )
            wps = ps.tile([1, QH], F32, tag="dpsum")
            for _ in range(32):
                nc.tensor.matmul(
                    wps[:, :128], ones[:], wsrc[:, :128], start=True, stop=True
                )

            units = []  # flat (slot, half, extent, tile) stream
            for s, e in enumerate(extents):
                for h in range(2):
                    for t in range(e):
                        units.append((s, h, e, t))
            T = len(units)
            st = {}  # live per-stream state
            pts = {}
            for i in range(T + 5):
                if i < T:
                    s, h, e, t = units[i]
                    if h == 0 and t == 0:  # slot input DMAs (prefetchable)
                        wq, wk, wv, wb = _widths(e)
                        w = wq + wk + wv + wb
                        it = sb.tile([D, wmax], BF16, tag="in", bufs=3)
                        if s == 0:
                            # K then bias then Q-half0: tile 0 starts ASAP
                            nc.sync.dma_start(
                                it[:, wq : wq + wk], in_d[s][:, wq : wq + wk]
                            )
                            nc.sync.dma_start(
                                it[:, wq + wk + wv : w],
                                in_d[s][:, wq + wk + wv :],
                            )
                            nc.sync.dma_start(it[:, :QH], in_d[s][:, :QH])
                            nc.sync.dma_start(
                                it[:, wq + wk : wq + wk + wv],
                                in_d[s][:, wq + wk : wq + wk + wv],
                            )
                            nc.sync.dma_start(it[:, QH:wq], in_d[s][:, QH:wq])
                        else:
                            nc.sync.dma_start(it[:, :w], in_d[s][:])
                        st[s] = (it[:, wq + wk + wv : w], it, wq, wk)
                    bias, it, wq, wk = st[s]
                    q0 = h * QH
                    sp = ps.tile([KT, QH], F32, tag="spsum", bufs=2)
                    for c in range(2):
                        nc.tensor.matmul(
                            sp[:, c * 512 : (c + 1) * 512],
                            it[:, wq + t * KT : wq + (t + 1) * KT],
                            it[:, q0 + c * 512 : q0 + (c + 1) * 512],
                            start=True,
                            stop=True,
                        )
                    pt = sb.tile([KT, QH], BF16, tag="pt", bufs=8)
                    nc.scalar.activation(
                        pt[:],
                        sp[:],
                        mybir.ActivationFunctionType.Exp,
                        bias=bias[:, t : t + 1],
                        scale=SCALE,
                    )
                    pts[i] = pt
                if i > 4:
                    s, h, e, t = units[i - 5]
                    _, it, wq, wk = st[s]
                    q0 = h * QH
                    if t == 0:  # new (slot, half) consumer stream
                        opsum = ps.tile([DV, QH], F32, tag="opsum")
                        dpsum = ps.tile([1, QH], F32, tag="dpsum")
                        if h == 0:
                            osb = sb.tile([DV, NQ], F16, tag="osb")
                            dsb = sb.tile([1, NQ], F16, tag="dsb")
                            st[(s, "out")] = (osb, dsb)
                        st[(s, "acc")] = (opsum, dpsum)
                        pair = quad_base = None
                        first_d = True
                    opsum, dpsum = st[(s, "acc")]
                    osb, dsb = st[(s, "out")]
                    for c in range(2):
                        nc.tensor.matmul(
                            opsum[:, c * 512 : (c + 1) * 512],
                            it[:, wq + wk + t * DV : wq + wk + (t + 1) * DV],
                            pts[i - 5][:, c * 512 : (c + 1) * 512],
                            start=(t == 0),
                            stop=(t == e - 1),
                        )
                    # denominator: DVE pair/quad sums, one ones-matmul
                    # per 4 tiles
                    dmm = None
                    if t % 2 == 1:
                        pair = sb.tile([KT, QH], BF16, tag="ps2", bufs=3)
                        nc.vector.tensor_tensor(
                            pair[:], pts[i - 6][:], pts[i - 5][:],
                            mybir.AluOpType.add,
                        )
                        if t % 4 == 3:
                            quad = sb.tile([KT, QH], BF16, tag="ps4", bufs=3)
                            nc.vector.tensor_tensor(
                                quad[:], quad_base[:], pair[:],
                                mybir.AluOpType.add,
                            )
                            dmm = quad
                        else:
                            quad_base = pair
                    if t == e - 1 and dmm is None:
                        r = e % 4
                        if r == 1:
                            dmm = pts[i - 5]
                        elif r == 2:
                            dmm = pair
                        elif r == 3:
                            tail = sb.tile([KT, QH], BF16, tag="ps4", bufs=3)
                            nc.vector.tensor_tensor(
                                tail[:], quad_base[:], pts[i - 5][:],
                                mybir.AluOpType.add,
                            )
                            dmm = tail
                    if dmm is not None:
                        for c in range(2):
                            nc.tensor.matmul(
                                dpsum[:, c * 512 : (c + 1) * 512],
                                ones[:],
                                dmm[:, c * 512 : (c + 1) * 512],
                                start=first_d,
                                stop=(t == e - 1),
                            )
                        first_d = False
                    if t == e - 1:  # half done: evacuate PSUM fast --
                        # chunk 0 on DVE, chunk 1 on ScalarE in parallel, so
                        # the next half's first PV (WAR on opsum) unblocks in
                        # one copy-time, not two
                        nc.vector.tensor_copy(
                            osb[:, q0 : q0 + 640], opsum[:, 0:640]
                        )
                        nc.scalar.copy(
                            osb[:, q0 + 640 : q0 + QH], opsum[:, 640:QH]
                        )
                        if i == T + 4:  # tail: ScalarE is idle, DVE busy
                            nc.scalar.copy(dsb[:, q0 : q0 + QH], dpsum[:])
                        else:
                            nc.vector.tensor_copy(
                                dsb[:, q0 : q0 + QH], dpsum[:]
                            )
                        nc.gpsimd.dma_start(
                            o_d[s][:, q0 : q0 + QH], osb[:, q0 : q0 + QH]
                        )
                        if h == 1:  # slot done
                            nc.gpsimd.dma_start(d_d[s][:], dsb[:])
    nc.compile()
    return nc


# ------------------------------------------------------------------- kernel
def kernel(queries, keys, values, valid_lens, _trace=False):
    global LAST_RESULT
    queries = np.asarray(queries, dtype=np.float32)
    keys = np.asarray(keys, dtype=np.float32)
    values = np.asarray(values, dtype=np.float32)
    valid_lens = np.asarray(valid_lens, dtype=np.int32)

    sizes = [int((int(l) + KT - 1) // KT) for l in valid_lens]
    extents, assign = _schedule(sizes)
    key = tuple(extents)
    if key not in _PROGRAM_CACHE:
        _PROGRAM_CACHE[key] = _build(extents)
    nc = _PROGRAM_CACHE[key]

    bf16 = ml_dtypes.bfloat16
    qT = np.ascontiguousarray(queries.transpose(0, 2, 1)).astype(bf16)  # [B,D,NQ]
    kT = np.ascontiguousarray(keys.transpose(0, 2, 1)).astype(bf16)  # [B,D,NK]
    vb = values.astype(bf16)  # [B, NK, DV]
    # bias column per (batch, tile-row): 0 where key position valid else -1e6
    pos = np.arange(NK, dtype=np.int32).reshape(NK // KT, KT)  # [tiles, 128]
    bias_all = np.where(
        pos[None] < valid_lens[:, None, None], np.float32(0.0), NEG
    ).astype(bf16)  # [B, tiles, 128]

    in_maps = []
    for c in range(NCORES):
        m = {}
        for s, e in enumerate(extents):
            seg = assign[c][s]
            wq, wk, wv, wb = _widths(e)
            it = np.zeros((D, wq + wk + wv + wb), bf16)
            it[:, wq + wk + wv :] = bf16(NEG)
            if seg is not None:
                b, t0, n = seg
                it[:, :wq] = qT[b]
                it[:, wq : wq + n * KT] = kT[b][:, t0 * KT : (t0 + n) * KT]
                # V [n*KT, DV] -> SBUF image [KT, n*DV] (k-within-tile major)
                it[:, wq + wk : wq + wk + n * DV] = (
                    vb[b][t0 * KT : (t0 + n) * KT]
                    .reshape(n, KT, DV)
                    .transpose(1, 0, 2)
                    .reshape(KT, n * DV)
                )
                it[:, wq + wk + wv : wq + wk + wv + n] = bias_all[b][
                    t0 : t0 + n
                ].T
            m[f"in{s}"] = it
        in_maps.append(m)

    res = run_bass_kernel_spmd(
        nc, in_maps, core_ids=list(range(NCORES)), trace=_trace
    )
    LAST_RESULT = res

    o_acc = np.zeros((B, DV, NQ), np.float32)
    d_acc = np.zeros((B, NQ), np.float32)
    for c in range(NCORES):
        for s in range(len(extents)):
            seg = assign[c][s]
            if seg is None:
                continue
            b = seg[0]
            o_acc[b] += res.results[c][f"o{s}"].astype(np.float32)
            d_acc[b] += res.results[c][f"d{s}"][0].astype(np.float32)

    out = (o_acc / d_acc[:, None, :]).transpose(0, 2, 1)
    return np.ascontiguousarray(out.astype(np.float32))


# revision 18
# speedup vs baseline: 1.2677x; 1.2677x over previous
"""Masked dot-product attention on 8 TRN2 NeuronCores (Bass/Tile).

Strategy (fixed problem shape B=16, NQ=NK=2048, D=DV=128):

* Work unit = one 128-key k-tile of one batch.  ceil(L_b/128) tiles per
  batch; tiles are distributed over 8 cores as SPMD "slots": every core
  runs S slots with fixed extents [e_0..e_{S-1}]; the host assigns each
  (core, slot) one contiguous (batch, k-range) segment at call time (an
  exact DFS packer minimizes executed tiles).  Segments of one batch may
  land on different cores/slots; the host sums the partial results.

* Device math per k-tile t, per q-half (1024 queries):
    S^T[k,q]  = K_tile @ Q^T            (bf16 in, fp32 PSUM)
    P^T[k,q]  = exp(S^T/sqrt(D)+bias_k) (one ScalarE op, bf16 out; bias
                is -1e6 for masked keys so P underflows to exactly 0)
    O^T[v,q] += V_tile^T-contraction    (bf16 matmul, fp32 PSUM accum)
    d[1,q]   += ones^T @ (P-quad-sums)  (P tiles pair+quad-summed on DVE;
                one ones-matmul per 4 tiles keeps the PE under the
                ScalarE exp roofline)
  bf16 everywhere on-device: fp16 hits slow paths on every engine
  (no FWL for LDWEIGHTS, ~20% slower matmul stream, slower ACT + DVE).

* All slot inputs ride ONE packed bf16 DRAM tensor [128, Q|K|V|bias]
  (single ~700ns DMA-issue instead of five); outputs o (fp16 numerator)
  and d (fp16 denominator) are per-slot; host accumulates fp32, divides.
"""

import math

import ml_dtypes
import numpy as np

import concourse.bass as bass  # noqa: F401
import concourse.mybir as mybir
import concourse.tile as tile
from concourse import bacc
from concourse.bass_utils import run_bass_kernel_spmd
from concourse.vector_clock import ScopedClock


class _FastExitTileContext(tile.TileContext):
    """TileContext whose exit skips the end-of-program semaphore recycling.

    The stock ``_drain_and_barrier`` emits a serialized clear of every
    allocated semaphore (~250 x ~35ns) plus a DMA-queue reset and a second
    all-engine barrier -- ~9us of pure epilogue on the measured critical
    path.  Those exist so the semaphores are reusable by a LATER kernel in
    the same NEFF execution or a re-execution of the same loaded NEFF;
    each kernel() call here loads a fresh NEFF (sems start zeroed), so the
    final drain + one barrier is all that is needed for correctness of
    this single-kernel program.
    """

    def _drain_and_barrier(self, tick_clock, wait_clock):
        drain_inst = self.nc.sync.drain()
        wait_clock.add_sem_waits(
            drain_inst.ins, ScopedClock({None: tick_clock.global_clock})
        )
        popped = self.nc._tile_sem_poison_stack.pop()
        assert popped is self._sem_poison

B, NQ, NK, D, DV = 16, 2048, 2048, 128, 128
NCORES = 8
KT = 128  # keys per k-tile (partition dim)
QH = 1024  # queries per q-half (PSUM sizing)
NEG = np.float32(-1.0e6)
SCALE = 1.0 / math.sqrt(D)

F32 = mybir.dt.float32
F16 = mybir.dt.float16
BF16 = mybir.dt.bfloat16

_PROGRAM_CACHE: dict[tuple, object] = {}
LAST_RESULT = None  # BassKernelResults of the most recent run (for test.py)


# ---------------------------------------------------------------- scheduling
def _gen_extents(total, maxpart, nleft, prefix):
    """Descending tuples of nleft positive ints summing to total."""
    if nleft == 1:
        if 1 <= total <= maxpart:
            yield prefix + [total]
        return
    lo = (total + nleft - 1) // nleft
    for e in range(min(maxpart, total - nleft + 1), lo - 1, -1):
        yield from _gen_extents(total - e, e, nleft - 1, prefix + [e])


def _pack_exact(sizes, bins, node_cap=200000):
    """Cut batches into contiguous parts placed one-per-bin (part<=bin).
    bins: descending extent list (8 copies per slot).  Returns list of
    (bin_idx, batch, n) or None.  DFS w/ memo on (bin_idx, rem-state)."""
    nb = len(bins)
    suffix = [0] * (nb + 1)
    for i in range(nb - 1, -1, -1):
        suffix[i] = suffix[i + 1] + bins[i]
    seen = set()
    out = []
    nodes = [0]

    def dfs(bi, rem):
        nodes[0] += 1
        if nodes[0] > node_cap:
            return False
        tot = sum(rem)
        if tot == 0:
            return True
        if bi == nb or tot > suffix[bi]:
            return False
        key = (bi, tuple(sorted(rem)))
        if key in seen:
            return False
        order = sorted(range(len(rem)), key=lambda b: -rem[b])
        tried = set()
        for b in order:
            r = rem[b]
            if r <= 0 or r in tried:
                continue
            tried.add(r)
            c = min(r, bins[bi])
            rem[b] -= c
            out.append((bi, b, c))
            if dfs(bi + 1, rem):
                return True
            out.pop()
            rem[b] += c
        if dfs(bi + 1, rem):  # leave bin empty
            return True
        seen.add(key)
        return False

    return list(out) if dfs(0, list(sizes)) else None


def _schedule(sizes, n_cores=NCORES):
    """Pick slot extents minimizing (executed tiles, slots); assign parts.
    Returns (extents descending, assign[core][slot] = (batch, t0, n)|None)."""
    total = sum(sizes)
    cap = (total + n_cores - 1) // n_cores
    best = None
    for tot in range(cap, cap + 4):
        for nslots in range(1, 6):
            if best is not None:
                break
            for ext in _gen_extents(tot, min(16, max(sizes)), nslots, []):
                bins = [e for e in ext for _ in range(n_cores)]
                parts = _pack_exact(sizes, bins)
                if parts is not None:
                    best = (ext, parts)
                    break
        if best is not None:
            break
    assert best is not None, "exact packer failed"
    ext, parts = best
    nslots = len(ext)
    assign = [[None] * nslots for _ in range(n_cores)]
    nxt = [0] * len(sizes)
    # DFS emits parts in bin order, so same-batch parts get increasing t0.
    for bi, b, n in sorted(parts, key=lambda p: p[0]):
        s, c = bi // n_cores, bi % n_cores
        assign[c][s] = (b, nxt[b], n)
        nxt[b] += n
    # ascending (big slot last -> short tail drain), but lead with the
    # second-smallest: slot0's compute must cover slot1+2's input DMAs
    order = sorted(range(nslots), key=lambda s: ext[s])
    if len(order) > 1:
        order[0], order[1] = order[1], order[0]
    ext_o = [ext[s] for s in order]
    assign_o = [[row[s] for s in order] for row in assign]
    return ext_o, assign_o


def _widths(e):
    """Packed input column layout: Q | K | V | bias (all bf16)."""
    return NQ, e * KT, e * DV, e


# ------------------------------------------------------------ device program
def _build(extents):
    # The Bass preamble clears the full 254-sem kernel range (serialized
    # sem-file sweep + NRT pseudo-barrier fence, ~3us).  Semaphores are
    # already zero at NEFF load, so skip it -- same argument as the
    # _FastExitTileContext epilogue skip.
    _g = bass.BassGpSimd
    _orig = (_g.sem_clear, _g.dma_reset, bass.Bass._nrt_pseudo_barrier)
    _g.sem_clear = lambda self, sem: None
    _g.dma_reset = lambda self, semaphore_range=None: None
    bass.Bass._nrt_pseudo_barrier = lambda self: None
    try:
        nc = bacc.Bacc()
    finally:
        _g.sem_clear, _g.dma_reset, bass.Bass._nrt_pseudo_barrier = _orig
    emax = max(extents)
    in_d, o_d, d_d = [], [], []
    for s, e in enumerate(extents):
        w = sum(_widths(e))
        in_d.append(nc.dram_tensor(f"in{s}", [D, w], BF16, kind="ExternalInput"))
        o_d.append(nc.dram_tensor(f"o{s}", [DV, NQ], F16, kind="ExternalOutput"))
        d_d.append(nc.dram_tensor(f"d{s}", [1, NQ], F16, kind="ExternalOutput"))

    wmax = sum(_widths(emax))
    with _FastExitTileContext(nc) as tc:
        with (
            tc.tile_pool(name="sb", bufs=2) as sb,
            tc.tile_pool(name="ps", bufs=1, space="PSUM") as ps,
        ):
            ones = sb.tile([KT, 1], BF16, tag="ones", bufs=1)
            nc.vector.memset(ones[:], 1.0)
            # warmup: exp ACT-table load + PE HAM ramp during input DMA
            wsrc = sb.tile([KT, 512], BF16, tag="warm", bufs=1)
            nc.vector.memset(wsrc[:], 0.0)
            wpt = sb.tile([KT, QH], BF16, tag="pt", bufs=8)
            nc.scalar.activation(
                wpt[:, :512], wsrc[:], mybir.ActivationFunctionType.Exp
            )
            wps = ps.tile([1, QH], F32, tag="dpsum")
            for _ in range(32):
                nc.tensor.matmul(
                    wps[:, :128], ones[:], wsrc[:, :128], start=True, stop=True
                )

            units = []  # flat (slot, half, extent, tile) stream
            for s, e in enumerate(extents):
                for h in range(2):
                    for t in range(e):
                        units.append((s, h, e, t))
            T = len(units)
            st = {}  # live per-stream state
            pts = {}
            for i in range(T + 4):
                if i < T:
                    s, h, e, t = units[i]
                    if h == 0 and t == 0:  # slot input DMAs (prefetchable)
                        wq, wk, wv, wb = _widths(e)
                        w = wq + wk + wv + wb
                        it = sb.tile([D, wmax], BF16, tag="in", bufs=3)
                        if s == 0:
                            # K then bias then Q-half0: tile 0 starts ASAP
                            nc.sync.dma_start(
                                it[:, wq : wq + wk], in_d[s][:, wq : wq + wk]
                            )
                            nc.sync.dma_start(
                                it[:, wq + wk + wv : w],
                                in_d[s][:, wq + wk + wv :],
                            )
                            nc.sync.dma_start(it[:, :QH], in_d[s][:, :QH])
                            nc.sync.dma_start(
                                it[:, wq + wk : wq + wk + wv],
                                in_d[s][:, wq + wk : wq + wk + wv],
                            )
                            nc.sync.dma_start(it[:, QH:wq], in_d[s][:, QH:wq])
                        else:
                            nc.sync.dma_start(it[:, :w], in_d[s][:])
                        st[s] = (it[:, wq + wk + wv : w], it, wq, wk)
                    bias, it, wq, wk = st[s]
                    q0 = h * QH
                    sp = ps.tile([KT, QH], F32, tag="spsum", bufs=2)
                    for c in range(2):
                        nc.tensor.matmul(
                            sp[:, c * 512 : (c + 1) * 512],
                            it[:, wq + t * KT : wq + (t + 1) * KT],
                            it[:, q0 + c * 512 : q0 + (c + 1) * 512],
                            start=True,
                            stop=True,
                        )
                    pt = sb.tile([KT, QH], BF16, tag="pt", bufs=8)
                    nc.scalar.activation(
                        pt[:],
                        sp[:],
                        mybir.ActivationFunctionType.Exp,
                        bias=bias[:, t : t + 1],
                        scale=SCALE,
                    )
                    pts[i] = pt
                if i > 3:
                    s, h, e, t = units[i - 4]
                    _, it, wq, wk = st[s]
                    q0 = h * QH
                    if t == 0:  # new (slot, half) consumer stream
                        opsum = ps.tile([DV, QH], F32, tag="opsum")
                        dpsum = ps.tile([1, QH], F32, tag="dpsum")
                        if h == 0:
                            osb = sb.tile([DV, NQ], F16, tag="osb")
                            dsb = sb.tile([1, NQ], F16, tag="dsb")
                            st[(s, "out")] = (osb, dsb)
                        st[(s, "acc")] = (opsum, dpsum)
                        pair = quad_base = None
                        first_d = True
                    opsum, dpsum = st[(s, "acc")]
                    osb, dsb = st[(s, "out")]
                    for c in range(2):
                        nc.tensor.matmul(
                            opsum[:, c * 512 : (c + 1) * 512],
                            it[:, wq + wk + t * DV : wq + wk + (t + 1) * DV],
                            pts[i - 4][:, c * 512 : (c + 1) * 512],
                            start=(t == 0),
                            stop=(t == e - 1),
                        )
                    # denominator: DVE pair/quad sums, one ones-matmul
                    # per 4 tiles
                    dmm = None
                    if t % 2 == 1:
                        pair = sb.tile([KT, QH], BF16, tag="ps2", bufs=3)
                        nc.vector.tensor_tensor(
                            pair[:], pts[i - 5][:], pts[i - 4][:],
                            mybir.AluOpType.add,
                        )
                        if t % 4 == 3:
                            quad = sb.tile([KT, QH], BF16, tag="ps4", bufs=3)
                            nc.vector.tensor_tensor(
                                quad[:], quad_base[:], pair[:],
                                mybir.AluOpType.add,
                            )
                            dmm = quad
                        else:
                            quad_base = pair
                    if t == e - 1 and dmm is None:
                        r = e % 4
                        if r == 1:
                            dmm = pts[i - 4]
                        elif r == 2:
                            dmm = pair
                        elif r == 3:
                            tail = sb.tile([KT, QH], BF16, tag="ps4", bufs=3)
                            nc.vector.tensor_tensor(
                                tail[:], quad_base[:], pts[i - 4][:],
                                mybir.AluOpType.add,
                            )
                            dmm = tail
                    if dmm is not None:
                        for c in range(2):
                            nc.tensor.matmul(
                                dpsum[:, c * 512 : (c + 1) * 512],
                                ones[:],
                                dmm[:, c * 512 : (c + 1) * 512],
                                start=first_d,
                                stop=(t == e - 1),
                            )
                        first_d = False
                    if t == e - 1:  # half done: evacuate PSUM fast --
                        # chunk 0 on DVE, chunk 1 on ScalarE in parallel, so
                        # the next half's first PV (WAR on opsum) unblocks in
                        # one copy-time, not two
                        nc.vector.tensor_copy(
                            osb[:, q0 : q0 + 512], opsum[:, 0:512]
                        )
                        nc.scalar.copy(
                            osb[:, q0 + 512 : q0 + QH], opsum[:, 512:QH]
                        )
                        if i == T + 3:  # tail: ScalarE is idle, DVE busy
                            nc.scalar.copy(dsb[:, q0 : q0 + QH], dpsum[:])
                        else:
                            nc.vector.tensor_copy(
                                dsb[:, q0 : q0 + QH], dpsum[:]
                            )
                        nc.gpsimd.dma_start(
                            o_d[s][:, q0 : q0 + QH], osb[:, q0 : q0 + QH]
                        )
                        if h == 1:  # slot done
                            nc.gpsimd.dma_start(d_d[s][:], dsb[:])
    nc.compile()
    return nc


# ------------------------------------------------------------------- kernel
def kernel(queries, keys, values, valid_lens, _trace=False):
    global LAST_RESULT
    queries = np.asarray(queries, dtype=np.float32)
    keys = np.asarray(keys, dtype=np.float32)
    values = np.asarray(values, dtype=np.float32)
    valid_lens = np.asarray(valid_lens, dtype=np.int32)

    sizes = [int((int(l) + KT - 1) // KT) for l in valid_lens]
    extents, assign = _schedule(sizes)
    key = tuple(extents)
    if key not in _PROGRAM_CACHE:
        _PROGRAM_CACHE[key] = _build(extents)
    nc = _PROGRAM_CACHE[key]

    bf16 = ml_dtypes.bfloat16
    qT = np.ascontiguousarray(queries.transpose(0, 2, 1)).astype(bf16)  # [B,D,NQ]
    kT = np.ascontiguousarray(keys.transpose(0, 2, 1)).astype(bf16)  # [B,D,NK]
    vb = values.astype(bf16)  # [B, NK, DV]
    # bias column per (batch, tile-row): 0 where key position valid else -1e6
    pos = np.arange(NK, dtype=np.int32).reshape(NK // KT, KT)  # [tiles, 128]
    bias_all = np.where(
        pos[None] < valid_lens[:, None, None], np.float32(0.0), NEG
    ).astype(bf16)  # [B, tiles, 128]

    in_maps = []
    for c in range(NCORES):
        m = {}
        for s, e in enumerate(extents):
            seg = assign[c][s]
            wq, wk, wv, wb = _widths(e)
            it = np.zeros((D, wq + wk + wv + wb), bf16)
            it[:, wq + wk + wv :] = bf16(NEG)
            if seg is not None:
                b, t0, n = seg
                it[:, :wq] = qT[b]
                it[:, wq : wq + n * KT] = kT[b][:, t0 * KT : (t0 + n) * KT]
                # V [n*KT, DV] -> SBUF image [KT, n*DV] (k-within-tile major)
                it[:, wq + wk : wq + wk + n * DV] = (
                    vb[b][t0 * KT : (t0 + n) * KT]
                    .reshape(n, KT, DV)
                    .transpose(1, 0, 2)
                    .reshape(KT, n * DV)
                )
                it[:, wq + wk + wv : wq + wk + wv + n] = bias_all[b][
                    t0 : t0 + n
                ].T
            m[f"in{s}"] = it
        in_maps.append(m)

    res = run_bass_kernel_spmd(
        nc, in_maps, core_ids=list(range(NCORES)), trace=_trace
    )
    LAST_RESULT = res

    o_acc = np.zeros((B, DV, NQ), np.float32)
    d_acc = np.zeros((B, NQ), np.float32)
    for c in range(NCORES):
        for s in range(len(extents)):
            seg = assign[c][s]
            if seg is None:
                continue
            b = seg[0]
            o_acc[b] += res.results[c][f"o{s}"].astype(np.float32)
            d_acc[b] += res.results[c][f"d{s}"][0].astype(np.float32)

    out = (o_acc / d_acc[:, None, :]).transpose(0, 2, 1)
    return np.ascontiguousarray(out.astype(np.float32))


# revision 19
# speedup vs baseline: 1.2929x; 1.0199x over previous
"""Masked dot-product attention on 8 TRN2 NeuronCores (Bass/Tile).

Strategy (fixed problem shape B=16, NQ=NK=2048, D=DV=128):

* Work unit = one 128-key k-tile of one batch.  ceil(L_b/128) tiles per
  batch; tiles are distributed over 8 cores as SPMD "slots": every core
  runs S slots with fixed extents [e_0..e_{S-1}]; the host assigns each
  (core, slot) one contiguous (batch, k-range) segment at call time (an
  exact DFS packer minimizes executed tiles).  Segments of one batch may
  land on different cores/slots; the host sums the partial results.

* Device math per k-tile t, per q-half (1024 queries):
    S^T[k,q]  = K_tile @ Q^T            (bf16 in, fp32 PSUM)
    P^T[k,q]  = exp(S^T/sqrt(D)+bias_k) (one ScalarE op, bf16 out; bias
                is -1e6 for masked keys so P underflows to exactly 0)
    O^T[v,q] += V_tile^T-contraction    (bf16 matmul, fp32 PSUM accum)
    d[1,q]   += ones^T @ (P-quad-sums)  (P tiles pair+quad-summed on DVE;
                one ones-matmul per 4 tiles keeps the PE under the
                ScalarE exp roofline)
  The slot/half/tile loops are flattened into one software-pipelined
  stream (S/exp run 4 units ahead of PV) so slot and half boundaries
  cost no ScalarE bubbles; half-end PSUM evacuation is split across
  VectorE and ScalarE so the next half's first PV unblocks quickly.

* All slot inputs ride ONE packed bf16 DRAM tensor [128, Q|K|V|bias]
  (one ~700ns DMA-issue instead of five); outputs o (fp16 numerator)
  and d (fp16 denominator) are per-slot; host accumulates fp32, divides.
  The stock TileContext exit and Bass preamble each serialize a ~250-
  semaphore clear (~3-9us); both are skipped (semaphores start zeroed
  at NEFF load and this is a single-kernel program).
"""

import math

import ml_dtypes
import numpy as np

import concourse.bass as bass  # noqa: F401
import concourse.mybir as mybir
import concourse.tile as tile
from concourse import bacc
from concourse.bass_utils import run_bass_kernel_spmd
from concourse.vector_clock import ScopedClock


class _FastExitTileContext(tile.TileContext):
    """TileContext whose exit skips the end-of-program semaphore recycling.

    The stock ``_drain_and_barrier`` emits a serialized clear of every
    allocated semaphore (~250 x ~35ns) plus a DMA-queue reset and a second
    all-engine barrier -- ~9us of pure epilogue on the measured critical
    path.  Those exist so the semaphores are reusable by a LATER kernel in
    the same NEFF execution or a re-execution of the same loaded NEFF;
    each kernel() call here loads a fresh NEFF (sems start zeroed), so the
    final drain + one barrier is all that is needed for correctness of
    this single-kernel program.
    """

    def _drain_and_barrier(self, tick_clock, wait_clock):
        drain_inst = self.nc.sync.drain()
        wait_clock.add_sem_waits(
            drain_inst.ins, ScopedClock({None: tick_clock.global_clock})
        )
        popped = self.nc._tile_sem_poison_stack.pop()
        assert popped is self._sem_poison

B, NQ, NK, D, DV = 16, 2048, 2048, 128, 128
NCORES = 8
KT = 128  # keys per k-tile (partition dim)
QH = 1024  # queries per q-half (PSUM sizing)
NEG = np.float32(-1.0e6)
SCALE = 1.0 / math.sqrt(D)

F32 = mybir.dt.float32
F16 = mybir.dt.float16
BF16 = mybir.dt.bfloat16

_PROGRAM_CACHE: dict[tuple, object] = {}
LAST_RESULT = None  # BassKernelResults of the most recent run (for test.py)


# ---------------------------------------------------------------- scheduling
def _gen_extents(total, maxpart, nleft, prefix):
    """Descending tuples of nleft positive ints summing to total."""
    if nleft == 1:
        if 1 <= total <= maxpart:
            yield prefix + [total]
        return
    lo = (total + nleft - 1) // nleft
    for e in range(min(maxpart, total - nleft + 1), lo - 1, -1):
        yield from _gen_extents(total - e, e, nleft - 1, prefix + [e])


def _pack_exact(sizes, bins, node_cap=200000):
    """Cut batches into contiguous parts placed one-per-bin (part<=bin).
    bins: descending extent list (8 copies per slot).  Returns list of
    (bin_idx, batch, n) or None.  DFS w/ memo on (bin_idx, rem-state)."""
    nb = len(bins)
    suffix = [0] * (nb + 1)
    for i in range(nb - 1, -1, -1):
        suffix[i] = suffix[i + 1] + bins[i]
    seen = set()
    out = []
    nodes = [0]

    def dfs(bi, rem):
        nodes[0] += 1
        if nodes[0] > node_cap:
            return False
        tot = sum(rem)
        if tot == 0:
            return True
        if bi == nb or tot > suffix[bi]:
            return False
        key = (bi, tuple(sorted(rem)))
        if key in seen:
            return False
        order = sorted(range(len(rem)), key=lambda b: -rem[b])
        tried = set()
        for b in order:
            r = rem[b]
            if r <= 0 or r in tried:
                continue
            tried.add(r)
            c = min(r, bins[bi])
            rem[b] -= c
            out.append((bi, b, c))
            if dfs(bi + 1, rem):
                return True
            out.pop()
            rem[b] += c
        if dfs(bi + 1, rem):  # leave bin empty
            return True
        seen.add(key)
        return False

    return list(out) if dfs(0, list(sizes)) else None


def _schedule(sizes, n_cores=NCORES):
    """Pick slot extents minimizing (executed tiles, slots); assign parts.
    Returns (extents descending, assign[core][slot] = (batch, t0, n)|None)."""
    total = sum(sizes)
    cap = (total + n_cores - 1) // n_cores
    best = None
    for tot in range(cap, cap + 4):
        for nslots in range(1, 6):
            if best is not None:
                break
            for ext in _gen_extents(tot, min(16, max(sizes)), nslots, []):
                bins = [e for e in ext for _ in range(n_cores)]
                parts = _pack_exact(sizes, bins)
                if parts is not None:
                    best = (ext, parts)
                    break
        if best is not None:
            break
    if best is None:
        # guaranteed-feasible fallback: two 16-wide slots hold any batch
        # (sizes are <= NK/KT = 16) one-per-bin across 16 bins
        ext = [16, 16]
        parts, bi = [], 0
        for b, r in enumerate(sizes):
            if r > 0:
                parts.append((bi, b, r))
                bi += 1
        best = (ext, parts)
    ext, parts = best
    nslots = len(ext)
    assign = [[None] * nslots for _ in range(n_cores)]
    nxt = [0] * len(sizes)
    # DFS emits parts in bin order, so same-batch parts get increasing t0.
    for bi, b, n in sorted(parts, key=lambda p: p[0]):
        s, c = bi // n_cores, bi % n_cores
        assign[c][s] = (b, nxt[b], n)
        nxt[b] += n
    # ascending (big slot last -> short tail drain), but lead with the
    # second-smallest: slot0's compute must cover slot1+2's input DMAs
    order = sorted(range(nslots), key=lambda s: ext[s])
    if len(order) > 1:
        order[0], order[1] = order[1], order[0]
    ext_o = [ext[s] for s in order]
    assign_o = [[row[s] for s in order] for row in assign]
    return ext_o, assign_o


def _widths(e):
    """Packed input column layout: Q | K | V | bias (all bf16)."""
    return NQ, e * KT, e * DV, e


# ------------------------------------------------------------ device program
def _build(extents):
    # The Bass preamble clears the full 254-sem kernel range (serialized
    # sem-file sweep + NRT pseudo-barrier fence, ~3us).  Semaphores are
    # already zero at NEFF load, so skip it -- same argument as the
    # _FastExitTileContext epilogue skip.
    _g = bass.BassGpSimd
    _orig = (_g.sem_clear, _g.dma_reset, bass.Bass._nrt_pseudo_barrier)
    _g.sem_clear = lambda self, sem: None
    _g.dma_reset = lambda self, semaphore_range=None: None
    bass.Bass._nrt_pseudo_barrier = lambda self: None
    try:
        nc = bacc.Bacc()
    finally:
        _g.sem_clear, _g.dma_reset, bass.Bass._nrt_pseudo_barrier = _orig
    emax = max(extents)
    in_d, o_d, d_d = [], [], []
    for s, e in enumerate(extents):
        w = sum(_widths(e))
        in_d.append(nc.dram_tensor(f"in{s}", [D, w], BF16, kind="ExternalInput"))
        o_d.append(nc.dram_tensor(f"o{s}", [DV, NQ], F16, kind="ExternalOutput"))
        d_d.append(nc.dram_tensor(f"d{s}", [1, NQ], F16, kind="ExternalOutput"))

    wmax = sum(_widths(emax))
    with _FastExitTileContext(nc) as tc:
        with (
            tc.tile_pool(name="sb", bufs=2) as sb,
            tc.tile_pool(name="ps", bufs=1, space="PSUM") as ps,
        ):
            ones = sb.tile([KT, 1], BF16, tag="ones", bufs=1)
            nc.vector.memset(ones[:], 1.0)
            # warmup: exp ACT-table load + PE HAM ramp during input DMA
            wsrc = sb.tile([KT, 512], BF16, tag="warm", bufs=1)
            nc.vector.memset(wsrc[:], 0.0)
            wpt = sb.tile([KT, QH], BF16, tag="pt", bufs=8)
            nc.scalar.activation(
                wpt[:, :512], wsrc[:], mybir.ActivationFunctionType.Exp
            )
            wps = ps.tile([1, QH], F32, tag="dpsum")
            for _ in range(32):
                nc.tensor.matmul(
                    wps[:, :128], ones[:], wsrc[:, :128], start=True, stop=True
                )

            units = []  # flat (slot, half, extent, tile) stream
            for s, e in enumerate(extents):
                for h in range(2):
                    for t in range(e):
                        units.append((s, h, e, t))
            T = len(units)
            st = {}  # live per-stream state
            pts = {}
            for i in range(T + 4):
                if i < T:
                    s, h, e, t = units[i]
                    if h == 0 and t == 0:  # slot input DMAs (prefetchable)
                        wq, wk, wv, wb = _widths(e)
                        w = wq + wk + wv + wb
                        it = sb.tile([D, wmax], BF16, tag="in", bufs=3)
                        if s == 0:
                            # K then bias then Q-half0: tile 0 starts ASAP
                            nc.sync.dma_start(
                                it[:, wq : wq + wk], in_d[s][:, wq : wq + wk]
                            )
                            nc.sync.dma_start(
                                it[:, wq + wk + wv : w],
                                in_d[s][:, wq + wk + wv :],
                            )
                            nc.sync.dma_start(it[:, :QH], in_d[s][:, :QH])
                            nc.sync.dma_start(
                                it[:, wq + wk : wq + wk + wv],
                                in_d[s][:, wq + wk : wq + wk + wv],
                            )
                            nc.sync.dma_start(it[:, QH:wq], in_d[s][:, QH:wq])
                        else:
                            nc.sync.dma_start(it[:, :w], in_d[s][:])
                        st[s] = (it[:, wq + wk + wv : w], it, wq, wk)
                    bias, it, wq, wk = st[s]
                    q0 = h * QH
                    sp = ps.tile([KT, QH], F32, tag="spsum", bufs=2)
                    for c in range(2):
                        nc.tensor.matmul(
                            sp[:, c * 512 : (c + 1) * 512],
                            it[:, wq + t * KT : wq + (t + 1) * KT],
                            it[:, q0 + c * 512 : q0 + (c + 1) * 512],
                            start=True,
                            stop=True,
                        )
                    pt = sb.tile([KT, QH], BF16, tag="pt", bufs=8)
                    nc.scalar.activation(
                        pt[:],
                        sp[:],
                        mybir.ActivationFunctionType.Exp,
                        bias=bias[:, t : t + 1],
                        scale=SCALE,
                    )
                    pts[i] = pt
                if i > 3:
                    s, h, e, t = units[i - 4]
                    _, it, wq, wk = st[s]
                    q0 = h * QH
                    if t == 0:  # new (slot, half) consumer stream
                        opsum = ps.tile([DV, QH], F32, tag="opsum")
                        dpsum = ps.tile([1, QH], F32, tag="dpsum")
                        if h == 0:
                            osb = sb.tile([DV, NQ], F16, tag="osb")
                            dsb = sb.tile([1, NQ], F16, tag="dsb")
                            st[(s, "out")] = (osb, dsb)
                        st[(s, "acc")] = (opsum, dpsum)
                        pair = quad_base = None
                        first_d = True
                    opsum, dpsum = st[(s, "acc")]
                    osb, dsb = st[(s, "out")]
                    for c in range(2):
                        nc.tensor.matmul(
                            opsum[:, c * 512 : (c + 1) * 512],
                            it[:, wq + wk + t * DV : wq + wk + (t + 1) * DV],
                            pts[i - 4][:, c * 512 : (c + 1) * 512],
                            start=(t == 0),
                            stop=(t == e - 1),
                        )
                    # denominator: DVE pair/quad sums, one ones-matmul
                    # per 4 tiles
                    dmm = None
                    if t % 2 == 1:
                        pair = sb.tile([KT, QH], BF16, tag="ps2", bufs=3)
                        nc.vector.tensor_tensor(
                            pair[:], pts[i - 5][:], pts[i - 4][:],
                            mybir.AluOpType.add,
                        )
                        if t % 4 == 3:
                            quad = sb.tile([KT, QH], BF16, tag="ps4", bufs=3)
                            nc.vector.tensor_tensor(
                                quad[:], quad_base[:], pair[:],
                                mybir.AluOpType.add,
                            )
                            dmm = quad
                        else:
                            quad_base = pair
                    if t == e - 1 and dmm is None:
                        r = e % 4
                        if r == 1:
                            dmm = pts[i - 4]
                        elif r == 2:
                            dmm = pair
                        elif r == 3:
                            tail = sb.tile([KT, QH], BF16, tag="ps4", bufs=3)
                            nc.vector.tensor_tensor(
                                tail[:], quad_base[:], pts[i - 4][:],
                                mybir.AluOpType.add,
                            )
                            dmm = tail
                    if dmm is not None:
                        for c in range(2):
                            nc.tensor.matmul(
                                dpsum[:, c * 512 : (c + 1) * 512],
                                ones[:],
                                dmm[:, c * 512 : (c + 1) * 512],
                                start=first_d,
                                stop=(t == e - 1),
                            )
                        first_d = False
                    if t == e - 1:  # half done: evacuate PSUM fast --
                        # chunk 0 on DVE, chunk 1 on ScalarE in parallel, so
                        # the next half's first PV (WAR on opsum) unblocks in
                        # one copy-time, not two
                        nc.vector.tensor_copy(
                            osb[:, q0 : q0 + 512], opsum[:, 0:512]
                        )
                        nc.scalar.copy(
                            osb[:, q0 + 512 : q0 + QH], opsum[:, 512:QH]
                        )
                        if i == T + 3:  # tail: ScalarE is idle, DVE busy
                            nc.scalar.copy(dsb[:, q0 : q0 + QH], dpsum[:])
                        else:
                            nc.vector.tensor_copy(
                                dsb[:, q0 : q0 + QH], dpsum[:]
                            )
                        nc.gpsimd.dma_start(
                            o_d[s][:, q0 : q0 + QH], osb[:, q0 : q0 + QH]
                        )
                        if h == 1:  # slot done
                            nc.gpsimd.dma_start(d_d[s][:], dsb[:])
    nc.compile()
    return nc


# ------------------------------------------------------------------- kernel
def kernel(queries, keys, values, valid_lens, _trace=False):
    global LAST_RESULT
    queries = np.asarray(queries, dtype=np.float32)
    keys = np.asarray(keys, dtype=np.float32)
    values = np.asarray(values, dtype=np.float32)
    valid_lens = np.asarray(valid_lens, dtype=np.int32)

    sizes = [int((int(l) + KT - 1) // KT) for l in valid_lens]
    extents, assign = _schedule(sizes)
    key = tuple(extents)
    if key not in _PROGRAM_CACHE:
        _PROGRAM_CACHE[key] = _build(extents)
    nc = _PROGRAM_CACHE[key]

    bf16 = ml_dtypes.bfloat16
    qT = np.ascontiguousarray(queries.transpose(0, 2, 1)).astype(bf16)  # [B,D,NQ]
    kT = np.ascontiguousarray(keys.transpose(0, 2, 1)).astype(bf16)  # [B,D,NK]
    vb = values.astype(bf16)  # [B, NK, DV]
    # bias column per (batch, tile-row): 0 where key position valid else -1e6
    pos = np.arange(NK, dtype=np.int32).reshape(NK // KT, KT)  # [tiles, 128]
    bias_all = np.where(
        pos[None] < valid_lens[:, None, None], np.float32(0.0), NEG
    ).astype(bf16)  # [B, tiles, 128]

    in_maps = []
    for c in range(NCORES):
        m = {}
        for s, e in enumerate(extents):
            seg = assign[c][s]
            wq, wk, wv, wb = _widths(e)
            it = np.zeros((D, wq + wk + wv + wb), bf16)
            it[:, wq + wk + wv :] = bf16(NEG)
            if seg is not None:
                b, t0, n = seg
                it[:, :wq] = qT[b]
                it[:, wq : wq + n * KT] = kT[b][:, t0 * KT : (t0 + n) * KT]
                # V [n*KT, DV] -> SBUF image [KT, n*DV] (k-within-tile major)
                it[:, wq + wk : wq + wk + n * DV] = (
                    vb[b][t0 * KT : (t0 + n) * KT]
                    .reshape(n, KT, DV)
                    .transpose(1, 0, 2)
                    .reshape(KT, n * DV)
                )
                it[:, wq + wk + wv : wq + wk + wv + n] = bias_all[b][
                    t0 : t0 + n
                ].T
            m[f"in{s}"] = it
        in_maps.append(m)

    res = run_bass_kernel_spmd(
        nc, in_maps, core_ids=list(range(NCORES)), trace=_trace
    )
    LAST_RESULT = res

    o_acc = np.zeros((B, DV, NQ), np.float32)
    d_acc = np.zeros((B, NQ), np.float32)
    for c in range(NCORES):
        for s in range(len(extents)):
            seg = assign[c][s]
            if seg is None:
                continue
            b = seg[0]
            o_acc[b] += res.results[c][f"o{s}"].astype(np.float32)
            d_acc[b] += res.results[c][f"d{s}"][0].astype(np.float32)

    out = (o_acc / d_acc[:, None, :]).transpose(0, 2, 1)
    return np.ascontiguousarray(out.astype(np.float32))
